# revision 21
# baseline (speedup 1.0000x reference)
"""BFP8 block quantize-dequantize for Trainium2 (Bass/Tile), 8-core data parallel.

Problem: x (8, 4096, 4096) f32. Each contiguous block of 16 elements (along the
flattened last dims) shares an exponent e = floor(log2(max|x|)); values are
quantized to signed 8-bit mantissas at scale 2^(e-7) and dequantized back.

Sharding: pure data parallel on the leading axis — core c processes x[c]
([4096, 4096] = 64 MiB in). No cross-core communication.

Per-core kernel (memory-bound target):
  - Output is stored as bf16: q in [-128,127] times a power-of-two scale is
    exactly representable in bfloat16 (8 significand bits), so the bf16 store
    is lossless and cuts HBM traffic from 128 MiB to 96 MiB per core. The
    host widens bf16 -> f32 with a bit shift (no arithmetic).
  - 128x4096 f32 tiles, multi-buffered; loads from SP HWDGE, stores from ACT
    HWDGE so the two directions ride separate queue sets.
  - The quantize q = sat_int8(round(x * rcp)) is split across engines so no
    single engine is the bottleneck (DVE alone would be ~30% over the DMA
    budget if it did both the reduce and the full quantize):
      * DVE (only engine with free-axis reduce / bitwise ops): abs-max
        reduce; exponent-field mask; fused multiply+sat-int8-convert on the
        first SPLIT fraction of columns.
      * Pool: the scale/rcp exponent subtracts (int32 tensor_tensor with
        broadcast consts); the f32 multiply x*rcp on the remaining columns
        (exact: rcp is a power of two); the full dequantize q * scale ->
        bf16 (exact in bf16 under any rounding).
      * Act: f32 -> int8 saturating convert of Pool's multiply output
        (verified bit-identical to DVE's fused convert, incl. RNE ties).
  - Exponent bit-math: for normal floats floor(log2(m)) is the exponent
    field, so scale = 2^(e-7) and rcp = 2^(7-e) are exact bit manipulations.
    No zero/denormal clamp: with the randn input spec the smallest block max
    of 16 gaussians is ~0.03, so bmax is always a normal float.
"""
import numpy as np

try:
    import concourse.bacc as bacc
except ImportError:  # pragma: no cover - fallback for bare environments
    import sys
    for _p in ("/opt/trn_rl_repo", "/root/.axon_site/_ro/trn_rl_repo"):
        if _p not in sys.path:
            sys.path.insert(0, _p)
    import concourse.bacc as bacc
import concourse.mybir as mybir
import concourse.tile as tile
from concourse.bass_utils import run_bass_kernel_spmd

N_CORES = 8
P = 128                      # SBUF partitions
ROWS, COLS = 4096, 4096      # per-core shard
BLK = 16                     # elements sharing one exponent
MBITS_M1 = 7                 # mantissa_bits - 1
EXP_MASK = 0x7F800000

TILE_F = 4096                # f32 elements per partition per steady-state tile
# ramp tiles: small at the edges so the pipeline fills fast and the final
# tile's load->reduce->quantize->dequantize->store chain drains fast
TAPER_FRONT = [256, 256, 512, 1024, 2048]   # sums to 4096
TAPER_BACK = [2048, 1024, 512, 256, 256]
BUFS = 5
SPLIT_NUM, SPLIT_DEN = 90, 256  # DVE's share of quantize columns (in blocks)


def _schedule():
    total_f = ROWS * COLS // P
    end = sum(TAPER_FRONT) + sum(TAPER_BACK)
    mid = total_f - end
    assert mid % TILE_F == 0
    return TAPER_FRONT + [TILE_F] * (mid // TILE_F) + TAPER_BACK


def _split(f):
    """DVE-share column count for a tile of f columns (multiple of BLK)."""
    return (f * SPLIT_NUM // SPLIT_DEN) // BLK * BLK


def build(reps=1):
    nc = bacc.Bacc()
    x = nc.dram_tensor("x", [ROWS, COLS], mybir.dt.float32, kind="ExternalInput")
    out = nc.dram_tensor("out", [ROWS, COLS], mybir.dt.bfloat16, kind="ExternalOutput")

    sched = _schedule()
    offs = [0]
    for f in sched:
        offs.append(offs[-1] + P * f)
    assert offs[-1] == ROWS * COLS
    xflat = x[:].rearrange("r c -> (r c)")
    outflat = out[:].rearrange("r c -> (r c)")

    with tile.TileContext(nc) as tc:
        with tc.tile_pool(name="sbuf", bufs=BUFS) as pool:
            # [P,1] int32 broadcast constants for the Pool-side exponent
            # subtracts (Pool has no tensor_scalar / bitwise / integer max).
            c7 = pool.tile([P, 1], mybir.dt.int32, tag="c7")
            nc.vector.memset(c7[:], MBITS_M1 << 23)
            c254 = pool.tile([P, 1], mybir.dt.int32, tag="c254")
            nc.vector.memset(c254[:], 254 << 23)
            for t, f in [(t, f) for _ in range(reps) for t, f in enumerate(sched)]:
                nb = f // BLK
                s = _split(f)
                sb_ = s // BLK
                xt = pool.tile([P, f], mybir.dt.float32, tag="x")
                nc.sync.dma_start(xt[:], xflat[offs[t]:offs[t + 1]].rearrange("(p f) -> p f", p=P))
                x3 = xt[:].rearrange("p (b k) -> p b k", k=BLK)

                # block max|x|  (free-axis reduce: DVE only)
                bmax = pool.tile([P, nb], mybir.dt.float32, tag="bmax")
                nc.vector.tensor_reduce(
                    bmax[:], x3, axis=mybir.AxisListType.X,
                    op=mybir.AluOpType.max, apply_absolute_value=True,
                )
                #   expb   = bmax_bits & EXP_MASK   [DVE; == bits of 2^e]
                #   scaleb = expb - (7<<23)         [Pool; = bits of 2^(e-7)]
                #   rcpb   = (254<<23) - scaleb     [Pool; = bits of 2^(7-e)]
                expb = pool.tile([P, nb], mybir.dt.int32, tag="expb")
                nc.vector.tensor_scalar(
                    expb[:], bmax[:].bitcast(mybir.dt.int32),
                    scalar1=EXP_MASK, scalar2=None,
                    op0=mybir.AluOpType.bitwise_and,
                )
                scaleb = pool.tile([P, nb], mybir.dt.int32, tag="scaleb")
                nc.gpsimd.tensor_tensor(
                    scaleb[:], expb[:], c7[:].broadcast_to((P, nb)),
                    op=mybir.AluOpType.subtract,
                )
                rcpb = pool.tile([P, nb], mybir.dt.int32, tag="rcpb")
                nc.gpsimd.tensor_tensor(
                    rcpb[:], c254[:].broadcast_to((P, nb)), scaleb[:],
                    op=mybir.AluOpType.subtract,
                )
                scale_b = scaleb[:].bitcast(mybir.dt.float32).unsqueeze(2).broadcast_to((P, nb, BLK))
                rcp_b = rcpb[:].bitcast(mybir.dt.float32).unsqueeze(2).broadcast_to((P, nb, BLK))

                # quantize: q = sat_int8(round(x * rcp)), split DVE | Pool+Act
                q = pool.tile([P, f], mybir.dt.int8, tag="q", bufs=4)
                nc.vector.tensor_tensor(
                    q[:, :s].rearrange("p (b k) -> p b k", k=BLK),
                    x3[:, :sb_], rcp_b[:, :sb_], op=mybir.AluOpType.mult,
                )
                tt = pool.tile([P, f - s], mybir.dt.float32, tag="tt", bufs=3)
                nc.gpsimd.tensor_tensor(
                    tt[:].rearrange("p (b k) -> p b k", k=BLK),
                    x3[:, sb_:], rcp_b[:, sb_:], op=mybir.AluOpType.mult,
                )
                nc.scalar.copy(q[:, s:], tt[:])

                # dequantize: out = q * scale  (exact in bf16: |q| <= 128 fits
                # 8 significand bits and scale is a power of two)
                deq = pool.tile([P, f], mybir.dt.bfloat16, tag="deq", bufs=4)
                nc.gpsimd.tensor_tensor(
                    deq[:].rearrange("p (b k) -> p b k", k=BLK),
                    q[:].rearrange("p (b k) -> p b k", k=BLK),
                    scale_b, op=mybir.AluOpType.mult,
                )
                nc.scalar.dma_start(
                    outflat[offs[t]:offs[t + 1]].rearrange("(p f) -> p f", p=P), deq[:])
    nc.finalize()
    return nc


_NC_CACHE = {}


def _get_nc(reps=1):
    if reps not in _NC_CACHE:
        _NC_CACHE[reps] = build(reps)
    return _NC_CACHE[reps]


def _bf16_to_f32(a: np.ndarray) -> np.ndarray:
    """Widen bf16 -> f32 by bit shift (exact, no arithmetic)."""
    u = np.asarray(a).view(np.uint16).astype(np.uint32) << 16
    return u.view(np.float32)


def kernel(x: np.ndarray) -> np.ndarray:
    x = np.asarray(x)
    assert x.shape == (N_CORES, ROWS, COLS) and x.dtype == np.float32, (x.shape, x.dtype)
    nc = _get_nc()
    in_maps = [{"x": np.ascontiguousarray(x[c])} for c in range(N_CORES)]
    res = run_bass_kernel_spmd(nc, in_maps, core_ids=list(range(N_CORES)))
    return np.stack([_bf16_to_f32(r["out"]) for r in res.results], axis=0)


# revision 25
# speedup vs baseline: 1.3111x; 1.3111x over previous
"""BFP8 block quantize-dequantize for Trainium2 (Bass/Tile), 8-core data parallel.

Problem: x (8, 4096, 4096) f32. Each contiguous block of 16 elements (along the
flattened last dims) shares an exponent e = floor(log2(max|x|)); values are
quantized to signed 8-bit mantissas at scale 2^(e-7) and dequantized back.

Sharding: pure data parallel on the leading axis — core c processes x[c]
([4096, 4096] = 64 MiB in). No cross-core communication.

Per-core kernel (memory-bound target):
  - Output is stored as bf16: q in [-128,127] times a power-of-two scale is
    exactly representable in bfloat16 (8 significand bits), so the bf16 store
    is lossless and cuts HBM traffic from 128 MiB to 96 MiB per core. The
    host widens bf16 -> f32 with a bit shift (no arithmetic).
  - 128x4096 f32 tiles, multi-buffered; loads from SP HWDGE, stores from ACT
    HWDGE so the two directions ride separate queue sets.
  - The quantize q = sat_int8(round(x * rcp)) is split across engines so no
    single engine is the bottleneck (DVE alone would be ~30% over the DMA
    budget if it did both the reduce and the full quantize):
      * DVE (only engine with free-axis reduce / bitwise ops): abs-max
        reduce; exponent-field mask; fused multiply+sat-int8-convert on the
        first SPLIT fraction of columns.
      * Pool: the scale/rcp exponent subtracts (int32 tensor_tensor with
        broadcast consts); the f32 multiply x*rcp on the remaining columns
        (exact: rcp is a power of two); the full dequantize q * scale ->
        bf16 (exact in bf16 under any rounding).
      * Act: f32 -> int8 saturating convert of Pool's multiply output
        (verified bit-identical to DVE's fused convert, incl. RNE ties).
  - Exponent bit-math: for normal floats floor(log2(m)) is the exponent
    field, so scale = 2^(e-7) and rcp = 2^(7-e) are exact bit manipulations.
    No zero/denormal clamp: with the randn input spec the smallest block max
    of 16 gaussians is ~0.03, so bmax is always a normal float.
"""
import numpy as np

try:
    import concourse.bacc as bacc
except ImportError:  # pragma: no cover - fallback for bare environments
    import sys
    for _p in ("/opt/trn_rl_repo", "/root/.axon_site/_ro/trn_rl_repo"):
        if _p not in sys.path:
            sys.path.insert(0, _p)
    import concourse.bacc as bacc
import concourse.mybir as mybir
import concourse.tile as tile
from concourse.bass_utils import run_bass_kernel_spmd

N_CORES = 8
P = 128                      # SBUF partitions
ROWS, COLS = 4096, 4096      # per-core shard
BLK = 16                     # elements sharing one exponent
MBITS_M1 = 7                 # mantissa_bits - 1
EXP_MASK = 0x7F800000

TILE_F = 4096                # f32 elements per partition per steady-state tile
# ramp tiles: small at the edges so the pipeline fills fast and the final
# tile's load->reduce->quantize->dequantize->store chain drains fast
TAPER_FRONT = [256, 256, 512, 1024, 2048]   # sums to 4096
TAPER_BACK = [2048, 1024, 512, 256, 256]
BUFS = 5
# Engine shares, calibrated to measured HW rates (Pool's software multiply
# runs at ~0.42 of its nominal rate, so big multiplies mostly stay on DVE):
SQ_NUM, SD_NUM, SPLIT_DEN = 192, 72, 256  # DVE's share of quantize / dequant


def _schedule():
    total_f = ROWS * COLS // P
    end = sum(TAPER_FRONT) + sum(TAPER_BACK)
    mid = total_f - end
    assert mid % TILE_F == 0
    return TAPER_FRONT + [TILE_F] * (mid // TILE_F) + TAPER_BACK


def _split(f, num):
    """DVE-share column count for a tile of f columns (multiple of BLK)."""
    return (f * num // SPLIT_DEN) // BLK * BLK


def build(reps=1):
    nc = bacc.Bacc()
    x = nc.dram_tensor("x", [ROWS, COLS], mybir.dt.float32, kind="ExternalInput")
    out = nc.dram_tensor("out", [ROWS, COLS], mybir.dt.bfloat16, kind="ExternalOutput")

    sched = _schedule()
    offs = [0]
    for f in sched:
        offs.append(offs[-1] + P * f)
    assert offs[-1] == ROWS * COLS
    xflat = x[:].rearrange("r c -> (r c)")
    outflat = out[:].rearrange("r c -> (r c)")

    with tile.TileContext(nc) as tc:
        with tc.tile_pool(name="sbuf", bufs=BUFS) as pool:
            # [P,1] int32 broadcast constants for the Pool-side exponent
            # subtracts (Pool has no tensor_scalar / bitwise / integer max).
            c7 = pool.tile([P, 1], mybir.dt.int32, tag="c7")
            nc.vector.memset(c7[:], MBITS_M1 << 23)
            c254 = pool.tile([P, 1], mybir.dt.int32, tag="c254")
            nc.vector.memset(c254[:], 254 << 23)
            for t, f in [(t, f) for _ in range(reps) for t, f in enumerate(sched)]:
                nb = f // BLK
                s = _split(f, SQ_NUM)
                sb_ = s // BLK
                sd = _split(f, SD_NUM)
                xt = pool.tile([P, f], mybir.dt.float32, tag="x")
                nc.sync.dma_start(xt[:], xflat[offs[t]:offs[t + 1]].rearrange("(p f) -> p f", p=P))
                x3 = xt[:].rearrange("p (b k) -> p b k", k=BLK)

                # block max|x|  (free-axis reduce: DVE only)
                bmax = pool.tile([P, nb], mybir.dt.float32, tag="bmax")
                nc.vector.tensor_reduce(
                    bmax[:], x3, axis=mybir.AxisListType.X,
                    op=mybir.AluOpType.max, apply_absolute_value=True,
                )
                #   expb   = bmax_bits & EXP_MASK   [DVE; == bits of 2^e]
                #   scaleb = expb - (7<<23)         [Pool; = bits of 2^(e-7)]
                #   rcpb   = (254<<23) - scaleb     [Pool; = bits of 2^(7-e)]
                expb = pool.tile([P, nb], mybir.dt.int32, tag="expb")
                nc.vector.tensor_scalar(
                    expb[:], bmax[:].bitcast(mybir.dt.int32),
                    scalar1=EXP_MASK, scalar2=None,
                    op0=mybir.AluOpType.bitwise_and,
                )
                scaleb = pool.tile([P, nb], mybir.dt.int32, tag="scaleb")
                nc.gpsimd.tensor_tensor(
                    scaleb[:], expb[:], c7[:].broadcast_to((P, nb)),
                    op=mybir.AluOpType.subtract,
                )
                rcpb = pool.tile([P, nb], mybir.dt.int32, tag="rcpb")
                nc.gpsimd.tensor_tensor(
                    rcpb[:], c254[:].broadcast_to((P, nb)), scaleb[:],
                    op=mybir.AluOpType.subtract,
                )
                scale_b = scaleb[:].bitcast(mybir.dt.float32).unsqueeze(2).broadcast_to((P, nb, BLK))
                rcp_b = rcpb[:].bitcast(mybir.dt.float32).unsqueeze(2).broadcast_to((P, nb, BLK))

                # quantize: q = sat_int8(round(x * rcp)), split DVE | Pool+Act
                q = pool.tile([P, f], mybir.dt.int8, tag="q", bufs=4)
                nc.vector.tensor_tensor(
                    q[:, :s].rearrange("p (b k) -> p b k", k=BLK),
                    x3[:, :sb_], rcp_b[:, :sb_], op=mybir.AluOpType.mult,
                )
                tt = pool.tile([P, f - s], mybir.dt.float32, tag="tt", bufs=3)
                nc.gpsimd.tensor_tensor(
                    tt[:].rearrange("p (b k) -> p b k", k=BLK),
                    x3[:, sb_:], rcp_b[:, sb_:], op=mybir.AluOpType.mult,
                )
                nc.scalar.copy(q[:, s:], tt[:])

                # dequantize: out = q * scale  (exact in bf16: |q| <= 128 fits
                # 8 significand bits and scale is a power of two), split
                # DVE | Pool like the quantize
                deq = pool.tile([P, f], mybir.dt.bfloat16, tag="deq", bufs=4)
                q3 = q[:].rearrange("p (b k) -> p b k", k=BLK)
                d3 = deq[:].rearrange("p (b k) -> p b k", k=BLK)
                sdb = sd // BLK
                nc.vector.tensor_tensor(
                    d3[:, :sdb], q3[:, :sdb], scale_b[:, :sdb],
                    op=mybir.AluOpType.mult,
                )
                nc.gpsimd.tensor_tensor(
                    d3[:, sdb:], q3[:, sdb:], scale_b[:, sdb:],
                    op=mybir.AluOpType.mult,
                )
                nc.scalar.dma_start(
                    outflat[offs[t]:offs[t + 1]].rearrange("(p f) -> p f", p=P), deq[:])
    nc.finalize()
    return nc


_NC_CACHE = {}


def _get_nc(reps=1):
    if reps not in _NC_CACHE:
        _NC_CACHE[reps] = build(reps)
    return _NC_CACHE[reps]


def _bf16_to_f32(a: np.ndarray) -> np.ndarray:
    """Widen bf16 -> f32 by bit shift (exact, no arithmetic)."""
    u = np.asarray(a).view(np.uint16).astype(np.uint32) << 16
    return u.view(np.float32)


def kernel(x: np.ndarray) -> np.ndarray:
    x = np.asarray(x)
    assert x.shape == (N_CORES, ROWS, COLS) and x.dtype == np.float32, (x.shape, x.dtype)
    nc = _get_nc()
    in_maps = [{"x": np.ascontiguousarray(x[c])} for c in range(N_CORES)]
    res = run_bass_kernel_spmd(nc, in_maps, core_ids=list(range(N_CORES)))
    return np.stack([_bf16_to_f32(r["out"]) for r in res.results], axis=0)
